# revision 17
# baseline (speedup 1.0000x reference)
"""Contrastive-loss kernel for Trainium2, SPMD over 8 NeuronCores.

The reference loss over x[N=4, S=4096, F=256] is, for pairs a>b with
D[a,b] = ||x[:,a]-x[:,b]||^2 (summed over batch and feature):

    loss = [ sum_{a>b, a-b>1} D[a,b] + sum_{b} relu(M - D[b+1,b]) ] / (S*(S-1)*1000)

Using symmetry of D (zero diagonal) this collapses to a streaming
computation that never materializes the [S,S] Gram matrix:

    sum_{a>b} D       = S * sum_t s[t] - sum_{n,f} c[n,f]^2
    s[t]              = sum_{n,f} x[n,t,f]^2
    c[n,f]            = sum_t x[n,t,f]
    D_sub[b]          = s[b] + s[b+1] - 2 * sum_{n,f} x[n,b+1,f]*x[n,b,f]
    numerator         = sum_{a>b} D - sum_b D_sub[b] + sum_b relu(M - D_sub[b])

Sharding: split the sequence dim into 8 chunks of 512 positions (+1 halo
row for the shifted product). Each core computes its partial c[1024],
sum_s, sum_D_sub and sum_hinge fully on device; the host gathers the 8
partial vectors/scalars and combines them (a ~1k-element sum of squares
plus a handful of adds) in float64.
"""

import numpy as np

import concourse.bass as bass
import concourse.tile as tile
from concourse import mybir
from concourse.bass_utils import run_bass_kernel_spmd

N, S, F = 4, 4096, 256
NCORES = 8
LOCAL = S // NCORES            # 512 positions per core
CH = LOCAL + 1                 # 513 chunk rows (1-row halo)
NBLK = LOCAL // 128            # 4 partition-blocks per core
NF = N * F                     # 1024
MARGIN = 60000.0
OUT_W = NF + 3                 # c partial (1024) + [sum_s, sum_D, sum_hinge]

_program = None
TRACE = False
LAST_RESULT = None


def _patch_sem_clear():
    """The walrus build in this container cannot encode
    EVENT_SEMAPHORE_RANGE_CLEAR ("ISA wrong length" in codegen). Replace the
    tail range-clear that TileContext emits via Bass.clear_and_free_semaphores
    with per-semaphore EventSemaphore writes of 0 (sem-wr-imm), which the
    compiler does support."""
    import bass_rust
    from concourse.bass import compact_to_ranges

    if getattr(bass.Bass, "_sem_clear_patched", False):
        return

    def clear_and_free_semaphores(self, sems):
        if not sems:
            return
        sem_nums = [s.num if hasattr(s, "num") else s for s in sems]
        for sem_range in compact_to_ranges(sem_nums):
            assert self._state.free_isdisjoint(sem_range)
            self.gpsimd.dma_reset(sem_range)
            for num in sem_range:
                h = bass_rust.SemaphoreHandle(num=num, name=f"clr{num}")
                bi = self.gpsimd.sem_inc(h, 1)
                upd = bi.ins.sync_info.on_update[0]
                upd.update_mode = "sem-wr-imm"
                upd.update_value = 0
        self._state.prepend_free_semaphores(sem_nums)
        for poison_set in self._tile_sem_poison_stack:
            poison_set.update(sem_nums)

    bass.Bass.clear_and_free_semaphores = clear_and_free_semaphores
    bass.Bass._sem_clear_patched = True


def _split_multi_waits(nc: bass.Bass) -> None:
    """The walrus build here encodes at most ONE sync wait per instruction.
    Hoist surplus waits into standalone wait-only EventSemaphore instructions
    placed immediately before the owner on the same engine queue — semantics
    are identical (same queue, in-order), and every instruction ends up with
    a single wait."""
    import bass_rust

    wid = 0
    for b in nc.m.functions[0].blocks:
        out = []
        changed = False
        for inst in b.instructions:
            si = inst.sync_info
            waits = list(si.on_wait) if si is not None else []
            if len(waits) > 1:
                changed = True
                for w in waits[:-1]:
                    ev = bass_rust.InstEventSemaphore(
                        name=f"WSPLIT-{wid}", engine=inst.engine, ins=[], outs=[]
                    )
                    wid += 1
                    ev.sync_info = bass_rust.SyncInfo(on_wait=[w], on_update=[])
                    out.append(ev)
                inst.sync_info = bass_rust.SyncInfo(
                    on_wait=[waits[-1]], on_update=list(si.on_update)
                )
            out.append(inst)
        if changed:
            b.instructions = out


def _build_program() -> bass.Bass:
    _patch_sem_clear()
    f32 = mybir.dt.float32
    nc = bass.Bass()
    xc = nc.declare_dram_parameter("xc", [N, CH, F], f32, isOutput=False)
    msk = nc.declare_dram_parameter("mask", [128, NBLK], f32, isOutput=False)
    out = nc.declare_dram_parameter("out", [1, OUT_W], f32, isOutput=True)

    with tile.TileContext(nc) as tc:
        with (
            tc.tile_pool(name="data", bufs=4) as data,
            tc.tile_pool(name="scratch", bufs=4) as scratch,
            tc.tile_pool(name="small", bufs=1) as small,
            tc.tile_pool(name="psum", bufs=1, space="PSUM") as psum,
        ):
            ones = small.tile([128, 1], f32)
            nc.vector.memset(ones, 1.0)
            marg = small.tile([128, 1], f32)
            nc.vector.memset(marg, MARGIN)
            mask_t = small.tile([128, NBLK], f32)
            nc.sync.dma_start(out=mask_t, in_=msk[:, :])
            # Stage the mask through DVE early: downstream consumers then
            # depend only on the DVE semaphore (one sync wait per op max).
            mask_v = small.tile([128, NBLK], f32)
            nc.vector.tensor_copy(mask_v, mask_t)

            sA = small.tile([128, NBLK], f32)   # s[t] per local position
            sB = small.tile([128, NBLK], f32)   # s[t+1]
            g2 = small.tile([128, NBLK], f32)   # -2 * x[t+1] . x[t]

            pc0 = psum.tile([1, 512], f32)      # c[:512] accumulator
            pc1 = psum.tile([1, 512], f32)      # c[512:] accumulator

            # A matmul instruction supports a single sync wait; the first
            # real matmul must wait on its DMA, so absorb the wait on the
            # ones-memset (DVE) into a throwaway PE op first.
            pwarm = psum.tile([1, 1], f32)
            nc.tensor.matmul(pwarm, ones, ones[:, 0:1], start=True, stop=True)

            xc_base = xc[:, :, :]
            for blk in range(NBLK):
                r0 = blk * 128
                # One DMA per block loading rows [r0, r0+128] AND the shifted
                # rows [r0+1, r0+129] into a single tile: a compute op may
                # carry only ONE sync wait, so both operands of the shifted
                # product must arrive via a single semaphore.
                # AB[p, n, 0:F] = x[n, r0+p, :], AB[p, n, F:2F] = x[n, r0+p+1, :]
                # (consecutive rows are contiguous in DRAM: one 2F-span each).
                AB = data.tile([128, N, 2 * F], f32, tag="AB")
                src = bass.AP(
                    tensor=xc_base.tensor,
                    offset=r0 * F,
                    ap=[[F, 128], [CH * F, N], [1, 2 * F]],
                )
                nc.sync.dma_start(out=AB, in_=src)
                A = AB[:, :, 0:F]
                B = AB[:, :, F : 2 * F]

                prod = scratch.tile([128, N, F], f32, tag="prod")
                sqA = scratch.tile([128, N, F], f32, tag="sqA")
                sqB = scratch.tile([128, N, F], f32, tag="sqB")

                nc.vector.tensor_mul(prod, A, B)
                nc.vector.reduce_sum(
                    out=g2[:, blk : blk + 1],
                    in_=prod.rearrange("r n f -> r (n f)"),
                    axis=mybir.AxisListType.X,
                )
                nc.scalar.activation(
                    out=sqA,
                    in_=A,
                    func=mybir.ActivationFunctionType.Square,
                    accum_out=sA[:, blk : blk + 1],
                )
                nc.scalar.activation(
                    out=sqB,
                    in_=B,
                    func=mybir.ActivationFunctionType.Square,
                    accum_out=sB[:, blk : blk + 1],
                )

                first, last = blk == 0, blk == NBLK - 1
                nc.tensor.matmul(pc0, ones, AB[:, 0:2, 0:F], start=first, stop=last)
                nc.tensor.matmul(pc1, ones, AB[:, 2:4, 0:F], start=first, stop=last)

            D = small.tile([128, NBLK], f32)
            Dm = small.tile([128, NBLK], f32)
            hinge = small.tile([128, NBLK], f32)
            Hm = small.tile([128, NBLK], f32)
            g2x = small.tile([128, NBLK], f32)
            nc.vector.tensor_add(Dm, sA, sB)
            nc.vector.tensor_scalar_mul(g2x, g2, 2.0)
            nc.vector.tensor_sub(D, Dm, g2x)
            # hinge = relu(MARGIN - D)
            nc.scalar.activation(
                out=hinge,
                in_=D,
                func=mybir.ActivationFunctionType.Relu,
                bias=marg[:, 0:1],
                scale=-1.0,
            )
            # mask kills the nonexistent b = S-1 pair on the last core
            nc.vector.tensor_mul(Dm, D, mask_v)
            nc.vector.tensor_mul(Hm, hinge, mask_v)

            fin = small.tile([128, 3], f32)
            nc.vector.reduce_sum(out=fin[:, 0:1], in_=sA, axis=mybir.AxisListType.X)
            nc.vector.reduce_sum(out=fin[:, 1:2], in_=Dm, axis=mybir.AxisListType.X)
            nc.vector.reduce_sum(out=fin[:, 2:3], in_=Hm, axis=mybir.AxisListType.X)
            pfin = psum.tile([1, 3], f32)
            nc.tensor.matmul(pfin, ones, fin, start=True, stop=True)

            ob = small.tile([1, OUT_W], f32)
            nc.scalar.copy(ob[:, 0:512], pc0)
            nc.scalar.copy(ob[:, 512:1024], pc1)
            nc.scalar.copy(ob[:, NF : NF + 3], pfin)
            nc.sync.dma_start(out=out[:, :], in_=ob)
    _split_multi_waits(nc)
    return nc


def _get_program() -> bass.Bass:
    global _program
    if _program is None:
        _program = _build_program()
    return _program


def kernel(**inputs) -> np.ndarray:
    global LAST_RESULT
    x = np.ascontiguousarray(np.asarray(inputs["x"], dtype=np.float32))
    assert x.shape == (N, S, F)
    nc = _get_program()

    in_maps = []
    for k in range(NCORES):
        t0 = k * LOCAL
        take = min(CH, S - t0)
        chunk = np.zeros((N, CH, F), dtype=np.float32)
        chunk[:, :take, :] = x[:, t0 : t0 + take, :]
        m = np.ones((128, NBLK), dtype=np.float32)
        if k == NCORES - 1:
            m[127, NBLK - 1] = 0.0
        in_maps.append({"xc": chunk, "mask": m})

    LAST_RESULT = run_bass_kernel_spmd(
        nc, in_maps, list(range(NCORES)), trace=TRACE
    )
    res = LAST_RESULT.results

    c = np.zeros(NF, dtype=np.float64)
    ssum = dsum = hsum = 0.0
    for r in res:
        o = r["out"][0].astype(np.float64)
        c += o[:NF]
        ssum += o[NF]
        dsum += o[NF + 1]
        hsum += o[NF + 2]
    gsum = float(np.sum(c * c))
    numerator = S * ssum - gsum - dsum + hsum
    loss = numerator / float(S * (S - 1) * 1000)
    return np.asarray(loss, dtype=np.float32)
